# revision 14
# baseline (speedup 1.0000x reference)
"""Adaptive-softmax (AdaSoftmaxGenerator) distributed Bass kernel for 8 trn2 cores.

Strategy: vocab-parallel. Each core owns a slice of every softmax group:
  head: 2500 of 20000 direct cols (+2 replicated cluster cols, +58 pad) = 2560
  tail1: 8500 of 68000 (+204 pad) = 8704
  tail0: 5000 of 40000 (+120 pad) = 5120
Total 16384 = 16 col-tiles of 1024, ordered [head | tail1 | tail0].
The head|tail1 boundary (2560) falls mid-tile-2; that tile's exp pass is
split into two half-tile ops with separate accumulator slots.

v5 design (v1 558us -> v2 419 -> v3 406 -> v4 266 -> v5):
  v4 profiling: PE at the full-clock floor (1024 fp8 DR MMs x 512cy = 223us
  @2.4GHz, gapless, HAM never throttled once collectives/fixups were gone),
  but ACT (256 exps + 256 accumulator reads = 235us active) overran the PE
  and left a 22us trailing-exp tail; startup burned 15us on serialized DMA
  triggers. v5:
  - Col-tiles widened to 1024 (NJ=16): exp ops drop to 136, amortizing the
    ~290ns fixed ACT overhead + 279ns accumulator-read per op.
    ACT ~= 186us < PE 223us. DVE drains halve to 64 x [128,2,1024].
  - Startup DMA triggers (xt8 / w0 / b0) issued on three different engines
    in parallel instead of serialized on sync.
  - As in v4: NO collectives, NO on-chip fixups. Planes (bf16 logits x16)
    stream to DRAM right after the bias-drain; raw per-(tile,bi) exp sums
    (136 slots) DMA out at the end; the HOST reduces partials across cores,
    forms the three per-row log-Z offsets (cluster correction included) and
    applies them during the unshard pass.
  - All matmuls fp8 DoubleRow; weights/bias host-scaled by 16; the exp pass
    un-scales via ACT `scale`; host divides the output by 16 on unshard.
Engine budget/core: PE 223us (wall), ACT ~186us, DVE ~154us, DMA ~155us.
"""

import sys
import types

sys.path.insert(0, "/opt/trn_rl_repo")

import numpy as np
import ml_dtypes

import concourse.bass as bass  # noqa: F401
import concourse.mybir as mybir
import concourse.tile as tile
from concourse import bacc
from concourse.bass_utils import run_bass_kernel_spmd

F32 = mybir.dt.float32
BF16 = mybir.dt.bfloat16
FP8 = mybir.dt.float8e4
AF = mybir.ActivationFunctionType
ALU = mybir.AluOpType

NCORES = 8
B = 1024
D = 1024
P = 128
NT = 1024  # col-tile width
H_OWN, T1_OWN, T0_OWN = 2500, 8500, 5000
HEAD_COLS, T1_COLS, T0_COLS = 2560, 8704, 5120  # padded per-core regions
NCOLS = HEAD_COLS + T1_COLS + T0_COLS  # 16384
NJ = 16
# groups: head = tiles 0,1 + first half of tile 2; tail1 = second half of
# tile 2 + tiles 3..10; tail0 = tiles 11..15.  (2560 = 2.5*NT; 11264 = 11*NT)
NSUB = (3, 9, 5)  # per-group sub-tile counts (tile 2 contributes a half to 0 and 1)
NSLOTS = 8 * sum(NSUB)  # 136
PAD_BIAS = -10000.0
WS = 16.0  # host-side weight/bias scale; undone by ACT scale + host divide

_cached_nc = None


def build():
    nc = bacc.Bacc(None, target_bir_lowering=False, debug=False)

    xt8_d = nc.declare_dram_parameter("xt8", [P, 8 * B], FP8, isOutput=False)
    wt8_d = nc.declare_dram_parameter("wt8", [NJ * P, 8 * NT], FP8, isOutput=False)
    bias_d = nc.declare_dram_parameter("biasc", [NJ * P, NT], BF16, isOutput=False)
    out_d = nc.declare_dram_parameter("out", [NJ * P, 8 * NT], BF16, isOutput=True)
    sums_d = nc.declare_dram_parameter("sums", [P, NSLOTS], F32, isOutput=True)

    xt8_r = xt8_d[:, :].rearrange("p (k b) -> p k b", k=8)

    def w_slice(j):
        return wt8_d[j * P : (j + 1) * P, :].rearrange("p (k c) -> p k c", k=8)

    def b_slice(j):
        return bias_d[j * P : (j + 1) * P, :]

    def out_slice(j):
        return out_d[j * P : (j + 1) * P, :].rearrange("p (bi c) -> p bi c", bi=8)

    def exp_ranges(j):
        """[(group, sub-index-within-group, col_lo, col_hi)] for tile j."""
        if j < 2:
            return [(0, j, 0, NT)]
        if j == 2:
            return [(0, 2, 0, 456), (1, 0, NT // 2, NT)]
        if j == 10:
            return [(1, 8, 0, 820)]
        if j < 11:
            return [(1, j - 2, 0, NT)]
        if j == 15:
            return [(2, 4, 0, 904)]
        return [(2, j - 11, 0, NT)]

    # pad trimming: real cols end mid-tile in tiles 2 (head), 10 (tail1)
    # and 15 (tail0); skip the all-pad remainder of the 512-wide mm chunks.
    MM_W = {(2, 0): 456, (10, 1): 308, (15, 1): 392}

    def wr_ranges(j):
        """written column ranges of tile j (drain / out-DMA extents)."""
        if j == 2:
            return [(0, 456), (512, 1024)]
        if j == 10:
            return [(0, 820)]
        if j == 15:
            return [(0, 904)]
        return [(0, NT)]

    with tile.TileContext(nc) as tc:
        with (
            tc.tile_pool(name="xt", bufs=1) as xt_pool,
            tc.tile_pool(name="w", bufs=3) as w_pool,
            tc.tile_pool(name="bias", bufs=3) as b_pool,
            tc.tile_pool(name="ps", bufs=2, space="PSUM") as ps_pool,
            tc.tile_pool(name="planes", bufs=4) as pl_pool,
            tc.tile_pool(name="exp", bufs=1) as ex_pool,
            tc.tile_pool(name="st", bufs=1) as st_pool,
        ):
            xt8_sb = xt_pool.tile([P, 8, B], FP8, tag="xt8", name="xt8")

            exp_t = ex_pool.tile([P, NT], FP8, tag="exp", name="exp")

            # per-group exp-sum accumulator slots
            sc = [
                st_pool.tile([P, 8 * NSUB[g]], F32, tag=f"sc{g}", name=f"sc{g}")
                for g in range(3)
            ]

            def do_tile(j, wb=None):
                if wb is None:
                    w_sb = w_pool.tile([P, 8, NT], FP8, tag="w", name="w")
                    nc.sync.dma_start(out=w_sb[:, :, :], in_=w_slice(j))
                    b_sb = b_pool.tile([P, NT], BF16, tag="bias", name="bias")
                    nc.sync.dma_start(out=b_sb[:, :], in_=b_slice(j))
                else:
                    w_sb, b_sb = wb
                bias_bc = (
                    b_sb[:, :]
                    .rearrange("p (one c) -> p one c", one=1)
                    .broadcast_to([P, 2, NT])
                )
                plane = pl_pool.tile([P, 8, NT], BF16, tag="pl", name=f"pl{j}")
                for bp in range(4):  # bi pairs share one 4-bank psum tile
                    psum = ps_pool.tile([P, 2, NT], F32, tag="ps", name="ps")
                    for half in range(2):
                        bi = 2 * bp + half
                        for k in range(4):
                            for ch in range(2):  # 512-wide psum-bank chunks
                                w = MM_W.get((j, ch), 512)
                                nc.tensor.matmul(
                                    psum[:, half, ch * 512 : ch * 512 + w],
                                    xt8_sb[
                                        :, 2 * k : 2 * k + 2, bi * P : (bi + 1) * P
                                    ],
                                    w_sb[
                                        :,
                                        2 * k : 2 * k + 2,
                                        ch * 512 : ch * 512 + w,
                                    ],
                                    start=(k == 0),
                                    stop=(k == 3),
                                    perf_mode=mybir.MatmulPerfMode.DoubleRow,
                                )
                    # plane16 = 16*(logit + bias) for both bi of the pair
                    for lo, hi in wr_ranges(j):
                        nc.vector.tensor_tensor(
                            plane[:, 2 * bp : 2 * bp + 2, lo:hi],
                            psum[:, :, lo:hi],
                            bias_bc[:, :, lo:hi],
                            op=ALU.add,
                        )
                        # stream the pair out immediately (host adds offsets);
                        # alternate gpsimd/sync so the gpsimd DGE drain ends
                        # before the kernel tail
                        (nc.gpsimd if bp % 2 == 0 else nc.sync).dma_start(
                            out=out_slice(j)[:, 2 * bp : 2 * bp + 2, lo:hi],
                            in_=plane[:, 2 * bp : 2 * bp + 2, lo:hi],
                        )
                    for half in range(2):
                        bi = 2 * bp + half
                        for g, sub, lo, hi in exp_ranges(j):
                            slot = bi * NSUB[g] + sub
                            nc.scalar.activation(
                                exp_t[:, lo:hi],
                                plane[:, bi, lo:hi],
                                AF.Exp,
                                scale=1.0 / WS,
                                accum_out=sc[g][:, slot : slot + 1],
                            )

            # ---- emission schedule
            # PE pre-warm: ~10 dummy MMs on zeroed scratch fill the PE-idle
            # window while the first w/x chunks stream in, so the HAM clock
            # gate is already at 8/8 (2.4GHz) when real MMs start.
            sw_scr = ex_pool.tile([P, 2, P], FP8, tag="swscr", name="swscr")
            sr_scr = ex_pool.tile([P, 2, 512], FP8, tag="srscr", name="srscr")
            nc.vector.memset(sw_scr[:, :, :], 0)
            nc.vector.memset(sr_scr[:, :, :], 0)
            ps_scr = ps_pool.tile([P, 2, NT], F32, tag="ps", name="ps")
            for _ in range(10):
                nc.tensor.matmul(
                    ps_scr[:, 0, 0:512],
                    sw_scr[:, :, :],
                    sr_scr[:, :, :],
                    start=True,
                    stop=True,
                    perf_mode=mybir.MatmulPerfMode.DoubleRow,
                )
            # startup DMAs k-chunked (256KB each) over two hw-DGE queue
            # engines so the first MM waits on one chunk only
            w0 = w_pool.tile([P, 8, NT], FP8, tag="w", name="w")
            b0 = b_pool.tile([P, NT], BF16, tag="bias", name="bias")
            for kk in range(4):
                nc.sync.dma_start(
                    out=xt8_sb[:, 2 * kk : 2 * kk + 2, :],
                    in_=xt8_r[:, 2 * kk : 2 * kk + 2, :],
                )
                nc.scalar.dma_start(
                    out=w0[:, 2 * kk : 2 * kk + 2, :],
                    in_=w_slice(0)[:, 2 * kk : 2 * kk + 2, :],
                )
            nc.sync.dma_start(out=b0[:, :], in_=b_slice(0))
            # per-group partial-sum DMAs fire as soon as each group's last
            # accumulator read retires (emitted on scalar: same engine, no
            # cross-engine wait); host does the cross-core reduction.
            SUM_OFF = (0, 8 * NSUB[0], 8 * (NSUB[0] + NSUB[1]))
            LAST_TILE = (2, 10, 15)
            do_tile(0, wb=(w0, b0))
            for j in range(1, NJ):
                do_tile(j)
                if j in LAST_TILE:
                    g = LAST_TILE.index(j)
                    off = SUM_OFF[g]
                    nc.scalar.dma_start(
                        out=sums_d[:, off : off + 8 * NSUB[g]], in_=sc[g][:, :]
                    )

    nc.compile()
    return nc


def get_nc():
    global _cached_nc
    if _cached_nc is None:
        _cached_nc = build()
    return _cached_nc


def make_in_maps(x, head_w, head_b, tail0_w, tail0_b, tail1_w, tail1_b):
    f8 = ml_dtypes.float8_e4m3fn
    x = np.asarray(x, np.float32)
    # xt8[p, k, b] = x[b, k*128+p]
    xt8 = (
        np.ascontiguousarray(x.T.reshape(8, P, B).transpose(1, 0, 2))
        .reshape(P, 8 * B)
        .astype(f8)
    )
    in_maps = []
    for c in range(NCORES):
        w_parts = [
            np.asarray(head_w[c * H_OWN : (c + 1) * H_OWN], np.float32),
            np.asarray(head_w[20000:20002], np.float32),
            np.zeros((HEAD_COLS - H_OWN - 2, D), np.float32),
            np.asarray(tail1_w[c * T1_OWN : (c + 1) * T1_OWN], np.float32),
            np.zeros((T1_COLS - T1_OWN, D), np.float32),
            np.asarray(tail0_w[c * T0_OWN : (c + 1) * T0_OWN], np.float32),
            np.zeros((T0_COLS - T0_OWN, D), np.float32),
        ]
        w = np.concatenate(w_parts, axis=0) * WS  # [NCOLS, D], 16x scaled
        # wt8[j, p, k, c] = w[j*NT+c, k*128+p]
        wt8 = (
            np.ascontiguousarray(w.reshape(NJ, NT, 8, P).transpose(0, 3, 2, 1))
            .reshape(NJ * P, 8 * NT)
            .astype(f8)
        )
        b_parts = [
            np.asarray(head_b[c * H_OWN : (c + 1) * H_OWN], np.float32),
            np.asarray(head_b[20000:20002], np.float32),
            np.full(HEAD_COLS - H_OWN - 2, PAD_BIAS, np.float32),
            np.asarray(tail1_b[c * T1_OWN : (c + 1) * T1_OWN], np.float32),
            np.full(T1_COLS - T1_OWN, PAD_BIAS, np.float32),
            np.asarray(tail0_b[c * T0_OWN : (c + 1) * T0_OWN], np.float32),
            np.full(T0_COLS - T0_OWN, PAD_BIAS, np.float32),
        ]
        bias = (np.concatenate(b_parts) * WS).astype(ml_dtypes.bfloat16)  # [NCOLS]
        # biasc[j*P+p, c] = bias[j*NT+c]  (partition-replicated only)
        bias_bc = np.ascontiguousarray(
            np.broadcast_to(bias.reshape(NJ, 1, NT), (NJ, P, NT))
        ).reshape(NJ * P, NT)
        in_maps.append({"xt8": xt8, "wt8": wt8, "biasc": bias_bc})
    return in_maps


def assemble(results):
    inv = 1.0 / WS
    prob = np.empty((B, 128000), np.float32)
    # per-group per-row exp sums, reduced across cores
    Z = np.zeros((3, B), np.float64)
    e_cl = None  # cluster exps (cols replicated on all cores)
    c_cl = None  # cluster logits
    for c in range(NCORES):
        o = results[c]["out"].astype(np.float32)  # [NJ*P, 8*NT]
        # logical[b, col]: b = bi*128+p, col = j*NT+ct
        o = o.reshape(NJ, P, 8, NT).transpose(2, 1, 0, 3).reshape(B, NCOLS) * inv
        prob[:, c * H_OWN : (c + 1) * H_OWN] = o[:, :H_OWN]
        prob[:, 60000 + c * T1_OWN : 60000 + (c + 1) * T1_OWN] = o[
            :, HEAD_COLS : HEAD_COLS + T1_OWN
        ]
        prob[:, 20000 + c * T0_OWN : 20000 + (c + 1) * T0_OWN] = o[
            :, HEAD_COLS + T1_COLS : HEAD_COLS + T1_COLS + T0_OWN
        ]
        if c == 0:
            c_cl = o[:, H_OWN : H_OWN + 2].astype(np.float64)  # [B, 2] logits
            e_cl = np.exp(c_cl)
        # sums[p, slot]: slot = group-major [g][bi*NSUB[g] + sub]; b = bi*128+p
        s = results[c]["sums"].astype(np.float64)  # [P, NSLOTS]
        off = 0
        for g in range(3):
            n = NSUB[g]
            blk = s[:, off : off + 8 * n].reshape(P, 8, n).sum(axis=2)
            Z[g] += blk.T.reshape(B)  # [bi, p] -> b = bi*128+p
            off += 8 * n
    # head: every core replicated the 2 cluster cols -> 8x over-count; the
    # planes are bit-identical across cores so subtract 7x exactly.
    Z[0] -= 7.0 * (e_cl[:, 0] + e_cl[:, 1])
    lzh = np.log(Z[0])
    lzt1 = np.log(Z[1])
    lzt0 = np.log(Z[2])
    off_head = (-lzh).astype(np.float32)
    off_t0 = (c_cl[:, 0] - lzh - lzt0).astype(np.float32)
    off_t1 = (c_cl[:, 1] - lzh - lzt1).astype(np.float32)
    prob[:, :20000] += off_head[:, None]
    prob[:, 20000:60000] += off_t0[:, None]
    prob[:, 60000:] += off_t1[:, None]
    return prob


def kernel(x, head_w, head_b, tail0_w, tail0_b, tail1_w, tail1_b):
    in_maps = make_in_maps(x, head_w, head_b, tail0_w, tail0_b, tail1_w, tail1_b)
    nc = get_nc()
    res = run_bass_kernel_spmd(nc, in_maps, core_ids=list(range(NCORES)))
    return assemble(res.results)


def run_traced(inputs):
    """Run with NTFF profiling; returns (prob, BassKernelResults)."""
    _hooks = types.ModuleType("antenv.axon_hooks")
    _hooks._hook = None
    _hooks.set_axon_ntff_profile_hook = lambda h: setattr(_hooks, "_hook", h)
    _hooks.get_axon_ntff_profile_hook = lambda: _hooks._hook
    sys.modules["antenv.axon_hooks"] = _hooks
    import antenv

    antenv.axon_hooks = _hooks
    from trn_agent_boot.trn_boot import _ntff_profile_via_ctypes

    _hooks.set_axon_ntff_profile_hook(
        _ntff_profile_via_ctypes("/opt/axon/libaxon_pjrt.so")
    )
    from concourse import bass_utils as _bu

    _bu.upload_artifacts = lambda tmpdir: tmpdir

    in_maps = make_in_maps(**inputs)
    nc = get_nc()
    res = run_bass_kernel_spmd(
        nc, in_maps, core_ids=list(range(NCORES)), trace=True
    )
    return assemble(res.results), res


# revision 15
# speedup vs baseline: 1.0223x; 1.0223x over previous
"""Adaptive-softmax (AdaSoftmaxGenerator) distributed Bass kernel for 8 trn2 cores.

Strategy: vocab-parallel. Each core owns a slice of every softmax group:
  head: 2500 of 20000 direct cols (+2 replicated cluster cols, +58 pad) = 2560
  tail1: 8500 of 68000 (+204 pad) = 8704
  tail0: 5000 of 40000 (+120 pad) = 5120
Total 16384 = 16 col-tiles of 1024, ordered [head | tail1 | tail0].
The head|tail1 boundary (2560) falls mid-tile-2; that tile's exp pass is
split into two half-tile ops with separate accumulator slots.

v5 design (v1 558us -> v2 419 -> v3 406 -> v4 266 -> v5):
  v4 profiling: PE at the full-clock floor (1024 fp8 DR MMs x 512cy = 223us
  @2.4GHz, gapless, HAM never throttled once collectives/fixups were gone),
  but ACT (256 exps + 256 accumulator reads = 235us active) overran the PE
  and left a 22us trailing-exp tail; startup burned 15us on serialized DMA
  triggers. v5:
  - Col-tiles widened to 1024 (NJ=16): exp ops drop to 136, amortizing the
    ~290ns fixed ACT overhead + 279ns accumulator-read per op.
    ACT ~= 186us < PE 223us. DVE drains halve to 64 x [128,2,1024].
  - Startup DMA triggers (xt8 / w0 / b0) issued on three different engines
    in parallel instead of serialized on sync.
  - As in v4: NO collectives, NO on-chip fixups. Planes (bf16 logits x16)
    stream to DRAM right after the bias-drain; raw per-(tile,bi) exp sums
    (136 slots) DMA out at the end; the HOST reduces partials across cores,
    forms the three per-row log-Z offsets (cluster correction included) and
    applies them during the unshard pass.
  - All matmuls fp8 DoubleRow; weights/bias host-scaled by 16; the exp pass
    un-scales via ACT `scale`; host divides the output by 16 on unshard.
Engine budget/core: PE 223us (wall), ACT ~186us, DVE ~154us, DMA ~155us.
"""

import sys
import types

sys.path.insert(0, "/opt/trn_rl_repo")

import numpy as np
import ml_dtypes

import concourse.bass as bass  # noqa: F401
import concourse.mybir as mybir
import concourse.tile as tile
from concourse import bacc
from concourse.bass_utils import run_bass_kernel_spmd

F32 = mybir.dt.float32
BF16 = mybir.dt.bfloat16
FP8 = mybir.dt.float8e4
AF = mybir.ActivationFunctionType
ALU = mybir.AluOpType

NCORES = 8
B = 1024
D = 1024
P = 128
NT = 1024  # col-tile width
H_OWN, T1_OWN, T0_OWN = 2500, 8500, 5000
HEAD_COLS, T1_COLS, T0_COLS = 2560, 8704, 5120  # padded per-core regions
NCOLS = HEAD_COLS + T1_COLS + T0_COLS  # 16384
NJ = 16
# groups: head = tiles 0,1 + first half of tile 2; tail1 = second half of
# tile 2 + tiles 3..10; tail0 = tiles 11..15.  (2560 = 2.5*NT; 11264 = 11*NT)
NSUB = (3, 9, 5)  # per-group sub-tile counts (tile 2 contributes a half to 0 and 1)
NSLOTS = 8 * sum(NSUB)  # 136
PAD_BIAS = -10000.0
WS = 16.0  # host-side weight/bias scale; undone by ACT scale + host divide

_cached_nc = None


def build():
    nc = bacc.Bacc(None, target_bir_lowering=False, debug=False)

    xt8_d = nc.declare_dram_parameter("xt8", [P, 8 * B], FP8, isOutput=False)
    wt8_d = nc.declare_dram_parameter("wt8", [NJ * P, 8 * NT], FP8, isOutput=False)
    bias_d = nc.declare_dram_parameter("biasc", [NJ * P, NT], BF16, isOutput=False)
    out_d = nc.declare_dram_parameter("out", [NJ * P, 8 * NT], BF16, isOutput=True)
    sums_d = nc.declare_dram_parameter("sums", [P, NSLOTS], F32, isOutput=True)

    xt8_r = xt8_d[:, :].rearrange("p (k b) -> p k b", k=8)

    def w_slice(j):
        return wt8_d[j * P : (j + 1) * P, :].rearrange("p (k c) -> p k c", k=8)

    def b_slice(j):
        return bias_d[j * P : (j + 1) * P, :]

    def out_slice(j):
        return out_d[j * P : (j + 1) * P, :].rearrange("p (bi c) -> p bi c", bi=8)

    def exp_ranges(j):
        """[(group, sub-index-within-group, col_lo, col_hi)] for tile j."""
        if j < 2:
            return [(0, j, 0, NT)]
        if j == 2:
            return [(0, 2, 0, 456), (1, 0, NT // 2, NT)]
        if j == 10:
            return [(1, 8, 0, 820)]
        if j < 11:
            return [(1, j - 2, 0, NT)]
        if j == 15:
            return [(2, 4, 0, 904)]
        return [(2, j - 11, 0, NT)]

    # pad trimming: real cols end mid-tile in tiles 2 (head), 10 (tail1)
    # and 15 (tail0); skip the all-pad remainder of the 512-wide mm chunks.
    MM_W = {(2, 0): 456, (10, 1): 308, (15, 1): 392}

    def wr_ranges(j):
        """written column ranges of tile j (drain / out-DMA extents)."""
        if j == 2:
            return [(0, 456), (512, 1024)]
        if j == 10:
            return [(0, 820)]
        if j == 15:
            return [(0, 904)]
        return [(0, NT)]

    with tile.TileContext(nc) as tc:
        with (
            tc.tile_pool(name="xt", bufs=1) as xt_pool,
            tc.tile_pool(name="w", bufs=3) as w_pool,
            tc.tile_pool(name="bias", bufs=3) as b_pool,
            tc.tile_pool(name="ps", bufs=2, space="PSUM") as ps_pool,
            tc.tile_pool(name="planes", bufs=4) as pl_pool,
            tc.tile_pool(name="exp", bufs=1) as ex_pool,
            tc.tile_pool(name="st", bufs=1) as st_pool,
        ):
            xt8_sb = xt_pool.tile([P, 8, B], FP8, tag="xt8", name="xt8")

            exp_t = ex_pool.tile([P, NT], FP8, tag="exp", name="exp")

            # per-group exp-sum accumulator slots
            sc = [
                st_pool.tile([P, 8 * NSUB[g]], F32, tag=f"sc{g}", name=f"sc{g}")
                for g in range(3)
            ]

            def do_tile(j, wb=None):
                if wb is None:
                    w_sb = w_pool.tile([P, 8, NT], FP8, tag="w", name="w")
                    nc.sync.dma_start(out=w_sb[:, :, :], in_=w_slice(j))
                    b_sb = b_pool.tile([P, NT], BF16, tag="bias", name="bias")
                    nc.sync.dma_start(out=b_sb[:, :], in_=b_slice(j))
                else:
                    w_sb, b_sb = wb
                bias_bc = (
                    b_sb[:, :]
                    .rearrange("p (one c) -> p one c", one=1)
                    .broadcast_to([P, 2, NT])
                )
                plane = pl_pool.tile([P, 8, NT], BF16, tag="pl", name=f"pl{j}")
                for bp in range(4):  # bi pairs share one 4-bank psum tile
                    psum = ps_pool.tile([P, 2, NT], F32, tag="ps", name="ps")
                    for half in range(2):
                        bi = 2 * bp + half
                        for k in range(4):
                            for ch in range(2):  # 512-wide psum-bank chunks
                                w = MM_W.get((j, ch), 512)
                                nc.tensor.matmul(
                                    psum[:, half, ch * 512 : ch * 512 + w],
                                    xt8_sb[
                                        :, 2 * k : 2 * k + 2, bi * P : (bi + 1) * P
                                    ],
                                    w_sb[
                                        :,
                                        2 * k : 2 * k + 2,
                                        ch * 512 : ch * 512 + w,
                                    ],
                                    start=(k == 0),
                                    stop=(k == 3),
                                    perf_mode=mybir.MatmulPerfMode.DoubleRow,
                                )
                    # plane16 = 16*(logit + bias) for both bi of the pair
                    for lo, hi in wr_ranges(j):
                        nc.vector.tensor_tensor(
                            plane[:, 2 * bp : 2 * bp + 2, lo:hi],
                            psum[:, :, lo:hi],
                            bias_bc[:, :, lo:hi],
                            op=ALU.add,
                        )
                        # stream the pair out immediately (host adds offsets);
                        # alternate gpsimd/sync so the gpsimd DGE drain ends
                        # before the kernel tail
                        (nc.gpsimd if bp % 2 == 0 else nc.sync).dma_start(
                            out=out_slice(j)[:, 2 * bp : 2 * bp + 2, lo:hi],
                            in_=plane[:, 2 * bp : 2 * bp + 2, lo:hi],
                        )
                    for half in range(2):
                        bi = 2 * bp + half
                        for g, sub, lo, hi in exp_ranges(j):
                            slot = bi * NSUB[g] + sub
                            nc.scalar.activation(
                                exp_t[:, lo:hi],
                                plane[:, bi, lo:hi],
                                AF.Exp,
                                scale=1.0 / WS,
                                accum_out=sc[g][:, slot : slot + 1],
                            )

            # ---- emission schedule: startup DMAs k-chunked (256KB each) over
            # two hw-DGE queue engines so the first MM waits on one chunk only
            w0 = w_pool.tile([P, 8, NT], FP8, tag="w", name="w")
            b0 = b_pool.tile([P, NT], BF16, tag="bias", name="bias")
            for kk in range(4):
                nc.sync.dma_start(
                    out=xt8_sb[:, 2 * kk : 2 * kk + 2, :],
                    in_=xt8_r[:, 2 * kk : 2 * kk + 2, :],
                )
                nc.scalar.dma_start(
                    out=w0[:, 2 * kk : 2 * kk + 2, :],
                    in_=w_slice(0)[:, 2 * kk : 2 * kk + 2, :],
                )
            nc.sync.dma_start(out=b0[:, :], in_=b_slice(0))
            # per-group partial-sum DMAs fire as soon as each group's last
            # accumulator read retires (emitted on scalar: same engine, no
            # cross-engine wait); host does the cross-core reduction.
            SUM_OFF = (0, 8 * NSUB[0], 8 * (NSUB[0] + NSUB[1]))
            LAST_TILE = (2, 10, 15)
            do_tile(0, wb=(w0, b0))
            for j in range(1, NJ):
                do_tile(j)
                if j in LAST_TILE:
                    g = LAST_TILE.index(j)
                    off = SUM_OFF[g]
                    nc.scalar.dma_start(
                        out=sums_d[:, off : off + 8 * NSUB[g]], in_=sc[g][:, :]
                    )

    nc.compile()
    return nc


def get_nc():
    global _cached_nc
    if _cached_nc is None:
        _cached_nc = build()
    return _cached_nc


def make_in_maps(x, head_w, head_b, tail0_w, tail0_b, tail1_w, tail1_b):
    f8 = ml_dtypes.float8_e4m3fn
    x = np.asarray(x, np.float32)
    # xt8[p, k, b] = x[b, k*128+p]
    xt8 = (
        np.ascontiguousarray(x.T.reshape(8, P, B).transpose(1, 0, 2))
        .reshape(P, 8 * B)
        .astype(f8)
    )
    in_maps = []
    for c in range(NCORES):
        w_parts = [
            np.asarray(head_w[c * H_OWN : (c + 1) * H_OWN], np.float32),
            np.asarray(head_w[20000:20002], np.float32),
            np.zeros((HEAD_COLS - H_OWN - 2, D), np.float32),
            np.asarray(tail1_w[c * T1_OWN : (c + 1) * T1_OWN], np.float32),
            np.zeros((T1_COLS - T1_OWN, D), np.float32),
            np.asarray(tail0_w[c * T0_OWN : (c + 1) * T0_OWN], np.float32),
            np.zeros((T0_COLS - T0_OWN, D), np.float32),
        ]
        w = np.concatenate(w_parts, axis=0) * WS  # [NCOLS, D], 16x scaled
        # wt8[j, p, k, c] = w[j*NT+c, k*128+p]
        wt8 = (
            np.ascontiguousarray(w.reshape(NJ, NT, 8, P).transpose(0, 3, 2, 1))
            .reshape(NJ * P, 8 * NT)
            .astype(f8)
        )
        b_parts = [
            np.asarray(head_b[c * H_OWN : (c + 1) * H_OWN], np.float32),
            np.asarray(head_b[20000:20002], np.float32),
            np.full(HEAD_COLS - H_OWN - 2, PAD_BIAS, np.float32),
            np.asarray(tail1_b[c * T1_OWN : (c + 1) * T1_OWN], np.float32),
            np.full(T1_COLS - T1_OWN, PAD_BIAS, np.float32),
            np.asarray(tail0_b[c * T0_OWN : (c + 1) * T0_OWN], np.float32),
            np.full(T0_COLS - T0_OWN, PAD_BIAS, np.float32),
        ]
        bias = (np.concatenate(b_parts) * WS).astype(ml_dtypes.bfloat16)  # [NCOLS]
        # biasc[j*P+p, c] = bias[j*NT+c]  (partition-replicated only)
        bias_bc = np.ascontiguousarray(
            np.broadcast_to(bias.reshape(NJ, 1, NT), (NJ, P, NT))
        ).reshape(NJ * P, NT)
        in_maps.append({"xt8": xt8, "wt8": wt8, "biasc": bias_bc})
    return in_maps


def assemble(results):
    inv = 1.0 / WS
    prob = np.empty((B, 128000), np.float32)
    # per-group per-row exp sums, reduced across cores
    Z = np.zeros((3, B), np.float64)
    e_cl = None  # cluster exps (cols replicated on all cores)
    c_cl = None  # cluster logits
    for c in range(NCORES):
        o = results[c]["out"].astype(np.float32)  # [NJ*P, 8*NT]
        # logical[b, col]: b = bi*128+p, col = j*NT+ct
        o = o.reshape(NJ, P, 8, NT).transpose(2, 1, 0, 3).reshape(B, NCOLS) * inv
        prob[:, c * H_OWN : (c + 1) * H_OWN] = o[:, :H_OWN]
        prob[:, 60000 + c * T1_OWN : 60000 + (c + 1) * T1_OWN] = o[
            :, HEAD_COLS : HEAD_COLS + T1_OWN
        ]
        prob[:, 20000 + c * T0_OWN : 20000 + (c + 1) * T0_OWN] = o[
            :, HEAD_COLS + T1_COLS : HEAD_COLS + T1_COLS + T0_OWN
        ]
        if c == 0:
            c_cl = o[:, H_OWN : H_OWN + 2].astype(np.float64)  # [B, 2] logits
            e_cl = np.exp(c_cl)
        # sums[p, slot]: slot = group-major [g][bi*NSUB[g] + sub]; b = bi*128+p
        s = results[c]["sums"].astype(np.float64)  # [P, NSLOTS]
        off = 0
        for g in range(3):
            n = NSUB[g]
            blk = s[:, off : off + 8 * n].reshape(P, 8, n).sum(axis=2)
            Z[g] += blk.T.reshape(B)  # [bi, p] -> b = bi*128+p
            off += 8 * n
    # head: every core replicated the 2 cluster cols -> 8x over-count; the
    # planes are bit-identical across cores so subtract 7x exactly.
    Z[0] -= 7.0 * (e_cl[:, 0] + e_cl[:, 1])
    lzh = np.log(Z[0])
    lzt1 = np.log(Z[1])
    lzt0 = np.log(Z[2])
    off_head = (-lzh).astype(np.float32)
    off_t0 = (c_cl[:, 0] - lzh - lzt0).astype(np.float32)
    off_t1 = (c_cl[:, 1] - lzh - lzt1).astype(np.float32)
    prob[:, :20000] += off_head[:, None]
    prob[:, 20000:60000] += off_t0[:, None]
    prob[:, 60000:] += off_t1[:, None]
    return prob


def kernel(x, head_w, head_b, tail0_w, tail0_b, tail1_w, tail1_b):
    in_maps = make_in_maps(x, head_w, head_b, tail0_w, tail0_b, tail1_w, tail1_b)
    nc = get_nc()
    res = run_bass_kernel_spmd(nc, in_maps, core_ids=list(range(NCORES)))
    return assemble(res.results)


def run_traced(inputs):
    """Run with NTFF profiling; returns (prob, BassKernelResults)."""
    _hooks = types.ModuleType("antenv.axon_hooks")
    _hooks._hook = None
    _hooks.set_axon_ntff_profile_hook = lambda h: setattr(_hooks, "_hook", h)
    _hooks.get_axon_ntff_profile_hook = lambda: _hooks._hook
    sys.modules["antenv.axon_hooks"] = _hooks
    import antenv

    antenv.axon_hooks = _hooks
    from trn_agent_boot.trn_boot import _ntff_profile_via_ctypes

    _hooks.set_axon_ntff_profile_hook(
        _ntff_profile_via_ctypes("/opt/axon/libaxon_pjrt.so")
    )
    from concourse import bass_utils as _bu

    _bu.upload_artifacts = lambda tmpdir: tmpdir

    in_maps = make_in_maps(**inputs)
    nc = get_nc()
    res = run_bass_kernel_spmd(
        nc, in_maps, core_ids=list(range(NCORES)), trace=True
    )
    return assemble(res.results), res
